# revision 95
# baseline (speedup 1.0000x reference)
"""MultiHeadAttention forward on 8 TRN2 NeuronCores (batch*head parallel).

Problem: S=2048, B=2, E=1024, H=16 heads, D=64. Each core handles one batch
(b = core//4) and 4 consecutive heads ((core%4)*4 ...), as 2 head-pairs.

This version (~262us trace-measured predecessor -> ~201us at the same clock
state): the softmax exp on the ACT is the hard floor (128 strips x ~1067ns
= 137us; exp must stay on ACT - DVE/GPSIMD have no exp, Schraudolph/fp8
tricks fail the 2e-2 budget because per-element error shows up ~1:1 in the
max-err/mean-ref metric), so the whole schedule is built to keep that exp
stream gap-free:

- Micro-task scheduling: all deferred PE work (V projection, the other
  pair's Q/K projections, out_proj) is chopped into ~0.4-1.7us tasks and
  interleaved one step-list per 2-strip step INSIDE the windows, sized to
  the per-step PE slack under the exp pace and ordered to meet each
  consumer's strip deadline. out_proj is deferred TWO windows so its attn
  dependency (the finalize chain) never blocks the in-order PE queue.
- Window-boundary software pipelining: the next window's first score pair
  + exps are hoisted before the current window's final P@V drain (carry
  dict hands the p-tiles across), so the exp stream crosses boundaries
  gap-free.
- Input DMA: one consolidated dma_start costs ~600ns sequencer issue and
  runs on ~one DMA engine (~25GB/s), so transfers are split 4-8 ways and
  spread over the SP/ACT HWDGE + GpSimd SWDGE queues; x quarter 0 lands
  column-major so V chunks project during the ramp. The exp ACT table is
  preloaded by a dummy activation emitted BEFORE the scalar-queue DMA
  issues (the issue backlog otherwise delays the first exp by ~20us).
- Tail: the last window's finalize avoids the ~6us recscr DRAM-bounce:
  DVE 32x32 block-transposes fold the PSUM denominator row into a strided
  column (read STRAIGHT from PSUM, in parallel with the ACT staging
  copies; fast [32,2,16] reciprocal), a reverse transpose rebuilds the
  contiguous 1/denom row, and a PE ones-matmul broadcasts it into PSUM
  for the normalize; the final out_proj runs a 4-deep matmul/cast pipeline
  (apsum + borrowed scores-PSUM ring) with casts split across DVE/ACT and
  out-DMAs split across both HWDGE queues.

NOTE the device clock varies run-to-run (exp slice 1038/1139/1247/1335ns
observed); compare timings via the EXP slice average, not wall ns.

Per-core program:
  Prefix: V chunks 0-3 + pair-0 Q/K t-quarter 0 only; everything else is
    in-window tasks. Q^T/K^T feature-major [f, s]; V natural [s, d] with a
    ones column (softmax denominator drops out of the P@V matmul).
  Windows: per head-pair, per t-quarter: row-packed K=64 score matmuls
    (heads at array rows 0-63/64-127 run concurrently), one ACT exp over
    the combined [128, 1024] PSUM strip producing fp16 P, P@V with [V|1]
    stationary deferred two strips, micro-tasks in the step slack.
  Finalize (interior, fully shadowed by the next window): PSUM->SBUF
    staging, DVE reciprocal over DMA-partition-shuffled denominators,
    stride-0 DMA broadcast from a DRAM bounce, DVE normalize multiply.
  Host: sums the 2x4 fp16 partials per batch in fp32, adds out_proj_bias.
"""
import os
import sys

if "/opt/trn_rl_repo" not in sys.path:
    sys.path.insert(0, "/opt/trn_rl_repo")

import ml_dtypes
import numpy as np

import concourse.bass as bass
import concourse.tile as tile
from concourse import mybir
from concourse.bass_utils import run_bass_kernel_spmd

_LDWOPT = os.environ.get("LDWOPT", "0") == "1"
if _LDWOPT:
    import concourse.bass_utils as _bu

    _orig_run_command = _bu.run_command

    def _run_command_ldwopt(argv, **kw):
        argv = ["--enable-ldw-opt=true" if a == "--enable-ldw-opt=false" else a
                for a in argv]
        return _orig_run_command(argv, **kw)

    _bu.run_command = _run_command_ldwopt

# BCAST mode: "dma" = stride-0 SBUF->SBUF DMA; "pe" = ones-matmul on the PE.
_BCAST = os.environ.get("BCAST", "dma")

S = 2048
B = 2
E = 1024
H = 16
D = 64
N_CORES = 8
F32 = mybir.dt.float32
F16 = mybir.dt.float16
F8 = mybir.dt.float8e4
F8NP = ml_dtypes.float8_e4m3
DR = mybir.MatmulPerfMode.DoubleRow
EXP = mybir.ActivationFunctionType.Exp
SCALING = float(D) ** -0.5
# fp8 weight scaling: wqk x64 (fold 1/64^2 into the exp scale), wv x16 (fold
# 1/16 into the ones/denominator column so P@V normalization cancels it).
QKW_SC = 64.0
VW_SC = 16.0

NSCH = S // 128   # 16 s-chunks
NSB = S // 512    # 4 s-blocks
NEC = E // 128    # 8 e-chunks


def _split_excess_waits(nc, limit=1):
    """This walrus build accepts at most 2 sync-wait commands per instruction;
    hoist excess waits onto preceding same-engine NOPs (queue order preserves
    semantics)."""
    ctr = 0
    for f in nc.m.functions:
        for blk in f.blocks:
            insts = blk.instructions
            if not any(
                i.sync_info and i.sync_info.on_wait and len(i.sync_info.on_wait) > limit
                for i in insts
            ):
                continue
            out = []
            for inst in insts:
                si = inst.sync_info
                if si and si.on_wait and len(si.on_wait) > limit:
                    waits = list(si.on_wait)
                    excess, keep = waits[:-limit], waits[-limit:]
                    for i in range(0, len(excess), limit):
                        ctr += 1
                        nop = mybir.InstNoOp(name=f"waitsplit-nop-{ctr}")
                        nop.engine = inst.engine
                        nop.sync_info = mybir.SyncInfo(
                            on_wait=excess[i : i + limit], on_update=[]
                        )
                        nc.register_instruction(nop, overwrite=True)
                        out.append(nop)
                    si.on_wait = keep
                out.append(inst)
            blk.instructions.clear()
            blk.instructions.extend(out)
    return nc


def _build_nc():
    nc = bass.Bass()
    xT = nc.dram_tensor("xT", [E, S], F16, kind="ExternalInput")
    wqkT = nc.dram_tensor("wqkT", [E, 512], F16, kind="ExternalInput")
    wvT = nc.dram_tensor("wvT", [E, 256], F16, kind="ExternalInput")
    woutT = nc.dram_tensor("woutT", [256, E], F16, kind="ExternalInput")
    bias_qk = nc.dram_tensor("bias_qk", [128, 4], F32, kind="ExternalInput")
    bias_v = nc.dram_tensor("bias_v", [1, 256], F32, kind="ExternalInput")
    outT = nc.dram_tensor("outT", [2, E, S], F16, kind="ExternalOutput")
    recscr = nc.dram_tensor("recscr", [2, 2, 512], F32, kind="Internal")

    with tile.TileContext(nc) as tc:
        with tc.tile_pool(name="wpool", bufs=1) as wpool, \
             tc.tile_pool(name="qkpool", bufs=1) as qkpool, \
             tc.tile_pool(name="vapool", bufs=1) as vapool, \
             tc.tile_pool(name="attnpool", bufs=1) as attnpool, \
             tc.tile_pool(name="ppool", bufs=4) as ppool, \
             tc.tile_pool(name="scpsum", bufs=2, space="PSUM") as scp, \
             tc.tile_pool(name="pvpsum", bufs=1, space="PSUM") as pvp:
            # ---- exp ACT table preload: the very first Scalar-queue
            # instruction, ahead of any DMA issues on that queue, so the
            # table load runs during the DMA-idle ramp.
            dummy = wpool.tile([1, 8], F16)
            nc.vector.memset(dummy, 0.0)
            nc.scalar.activation(dummy, dummy, EXP)
            # ---- constants / weights, consolidated into ONE dma_start per
            # tensor / x-quarter (SWDGE issue costs ~600ns each; dozens of
            # small issues serialize the sequencers for 25us+), spread over
            # four queues so issues and transfers overlap. x lands in
            # column-quarters: the strip loop starts after wqk + x[0:512].
            xt = wpool.tile([128, NEC, S], F16)
            wqk = wpool.tile([128, NEC, 512], F16)
            wv = wpool.tile([128, NEC, 256], F16)
            bqk = wpool.tile([128, 4], F32)
            bv = wpool.tile([128, 256], F32)
            wout = wpool.tile([128, 2, E], F16)

            def xq(quarter, e0, e1, nway=4):
                # a dma_start's descriptors run on ~one DMA engine
                # (~25GB/s); split the transfer for engine parallelism.
                cs = bass.ds(quarter * 512, 512)
                w = NEC // nway
                return [
                    (e0 if i % 2 == 0 else e1,
                     xt[:, bass.ds(i * w, w), cs],
                     xT[bass.ds(i * w * 128, w * 128), cs].rearrange(
                         "(c p) s -> p c s", p=128))
                    for i in range(nway)
                ]

            wqk_parts = [
                (nc.scalar, wqk[:, bass.ds(ec, 1), :],
                 wqkT[bass.ds(ec * 128, 128), :].rearrange(
                     "(c p) f -> p c f", p=128))
                for ec in range(NEC)
            ]
            # x quarter 0 lands column-major (cols 0-127 first) so V chunks
            # 0-3 can project during the warmup window while the rest of
            # quarter 0 still streams.
            xq0_parts = [
                (nc.sync, xt[:, :, bass.ds(c * 128, 128)],
                 xT[:, bass.ds(c * 128, 128)].rearrange(
                     "(c p) s -> p c s", p=128))
                for c in range(4)
            ]
            for eng, dst, src in (
                xq0_parts + wqk_parts +
                [(nc.gpsimd, wv[:, bass.ds(c0, 2), :],
                  wvT[bass.ds(c0 * 128, 256), :].rearrange(
                      "(c p) f -> p c f", p=128))
                 for c0 in (0, 2, 4, 6)] +
                [(nc.gpsimd, bqk, bias_qk[:, :]),
                 (nc.gpsimd, bv, bias_v[:, :].to_broadcast([128, 256]))] +
                xq(1, nc.sync, nc.scalar) +
                xq(2, nc.sync, nc.scalar) +
                xq(3, nc.sync, nc.scalar) +
                [(nc.gpsimd, wout,
                  woutT.rearrange("(c p) f -> p c f", p=128))]
            ):
                eng.dma_start(out=dst, in_=src)
            ones64 = wpool.tile([128, 64], F16)
            # named scope doubles as a compile-cache buster
            with nc.named_scope(f"init3_ldwopt{int(_LDWOPT)}_bc{_BCAST}"):
                nc.vector.memset(ones64, 1.0)
            onesbc = wpool.tile([1, 64], F16)
            nc.vector.tensor_copy(onesbc, ones64[0:1, :])

            # persistent activations
            qk = qkpool.tile([128, 4, S], F16)        # Q^T (chunks 0-1), K^T (2-3)
            # V natural + ones col, flattened per s-chunk to [4*65 + 68pad]
            # so each head's [V|1] stationary can be read as a 128-col AP
            # (full-width weights enable the PE fast weight load).
            va = vapool.tile([128, NSCH, 328], F16)
            attn = attnpool.tile([128, 2, S], F16)    # attn^T normalized

            def va_hd(i):
                return va[:, i, 0:260].rearrange("p (h c) -> p h c", h=4)

            nc.vector.memset(va[:, :, 260:328], 0.0)
            nc.vector.memset(
                va[:, :, 0:260].rearrange(
                    "p i (h c) -> p i h c", h=4)[:, :, :, 64:65], 1.0)

            with tc.tile_pool(name="apsum", bufs=2, space="PSUM") as apsum, \
                 tc.tile_pool(name="unpool", bufs=4) as unpool, \
                 tc.tile_pool(name="fpool", bufs=3) as fpool, \
                 tc.tile_pool(name="opool", bufs=6) as opool:

                # ---- HAM warm-up: the PE clock-gate needs ~3.4us of
                # sustained activity to reach 2.4 GHz. The PE is DMA-idle
                # until ~15us anyway, so free junk matmuls bridge the gap and
                # phase A starts at full clock instead of 1.2 GHz.
                junk = wpool.tile([128, 512], F16)
                nc.vector.memset(junk, 0.0)
                with nc.named_scope("ham_warmup"):
                    for _ in range(40):
                        jps = apsum.tile([128, 512], F32, tag="aps")
                        nc.tensor.matmul(
                            jps[0:64, :], ones64[:, 0:64], junk,
                            start=True, stop=True)

                def emit_qk(fc, sbs=range(NSB)):
                    with nc.named_scope(f"proj_qk{fc}"):
                        for sb in sbs:
                            ps = apsum.tile([128, 512], F32, tag="aps")
                            for ec in range(NEC):
                                nc.tensor.matmul(
                                    ps,
                                    wqk[:, ec, bass.ts(fc, 128)],
                                    xt[:, ec, bass.ts(sb, 512)],
                                    start=(ec == 0), stop=(ec == NEC - 1))
                            nc.vector.tensor_scalar(
                                out=qk[:, fc, bass.ts(sb, 512)], in0=ps,
                                scalar1=bqk[:, fc:fc + 1], scalar2=None,
                                op0=mybir.AluOpType.add)

                def emit_v(chunks=range(NSCH)):
                    with nc.named_scope("proj_v"):
                        for i in chunks:
                            ps = apsum.tile([128, 512], F32, tag="aps")
                            for ec in range(NEC):
                                nc.tensor.matmul(
                                    ps[:, 0:256],
                                    xt[:, ec, bass.ts(i, 128)],
                                    wv[:, ec, :],
                                    start=(ec == 0), stop=(ec == NEC - 1))
                            nc.vector.tensor_tensor(
                                out=va_hd(i)[:, :, 0:64],
                                in0=ps[:, 0:256].rearrange(
                                    "p (h d) -> p h d", h=4),
                                in1=bv.rearrange("p (h d) -> p h d", h=4),
                                op=mybir.AluOpType.add)

                def emit_oproj(pair, tq, final=False):
                    emit_oproj_fcs(pair, tq, range(NEC), final)

                # ---- micro-tasks: ~0.5-1.7us units of PE work interleaved
                # one-per-strip-step inside the windows (between the scores
                # and the P@V of a step), so the deferred work rides the
                # per-step PE slack instead of stalling the ACT at window
                # starts. A QK projection group is split into two halves
                # that share one open PSUM accumulation group; the two
                # halves MUST be adjacent in a task list (nothing else may
                # allocate from apsum in between).
                def t_qk(fc, sb):
                    def f():
                        emit_qk(fc, range(sb, sb + 1))
                    return f

                def t_qkh(fc, sb):
                    # one projection group split into two ~850ns halves that
                    # share an open PSUM accumulation group; schedule the
                    # halves in adjacent steps with no other apsum user in
                    # between.
                    st = {}

                    def first():
                        with nc.named_scope(f"tqk{fc}_{sb}a"):
                            st["ps"] = apsum.tile([128, 512], F32,
                                                  tag="aps", name="qkhps")
                            for ec in range(4):
                                nc.tensor.matmul(
                                    st["ps"],
                                    wqk[:, ec, bass.ts(fc, 128)],
                                    xt[:, ec, bass.ts(sb, 512)],
                                    start=(ec == 0), stop=False)

                    def second():
                        with nc.named_scope(f"tqk{fc}_{sb}b"):
                            ps = st["ps"]
                            for ec in range(4, NEC):
                                nc.tensor.matmul(
                                    ps,
                                    wqk[:, ec, bass.ts(fc, 128)],
                                    xt[:, ec, bass.ts(sb, 512)],
                                    start=False, stop=(ec == NEC - 1))
                            nc.vector.tensor_scalar(
                                out=qk[:, fc, bass.ts(sb, 512)], in0=ps,
                                scalar1=bqk[:, fc:fc + 1], scalar2=None,
                                op0=mybir.AluOpType.add)

                    return [first, second]

                def t_v(*chunks):
                    def f():
                        emit_v(chunks)
                    return f

                def t_oproj(pair, tq, final=False):
                    def half(fc0, fc1):
                        def f():
                            emit_oproj_fcs(pair, tq, range(fc0, fc1), final)
                        return f
                    return [half(0, 2), half(2, 4), half(4, 6), half(6, 8)]

                def emit_oproj_fcs(pair, tq, fcs, final=False, trange=None):
                    toff = tq * 512
                    t0, tlen = trange if trange else (0, 512)
                    tsl = bass.ds(toff + t0, tlen)
                    with nc.named_scope(f"oproj{pair}_{tq}"):
                        for fc in fcs:
                            if final and fc % 2 == 1:
                                # tail-only: borrow the idle scores PSUM ring
                                # (and below the freed P@V banks) for a
                                # 6-deep matmul/cast pipeline.
                                ps = scp.tile([128, 1024], F32, tag="sc",
                                              name="fops")[:, 0:tlen]
                            elif final and fc % 4 == 2:
                                ps = pvp.tile([128, 512], F32, tag="pvA",
                                              name="fopv")
                                ps = ps[:, 0:tlen]
                            else:
                                ps = apsum.tile([128, 512], F32, tag="aps",
                                                name="ops")
                                ps = ps[:, 0:tlen]
                            nc.tensor.matmul(
                                ps,
                                wout[:, pair, bass.ts(fc, 128)],
                                attn[:, pair, tsl],
                                start=True, stop=True)
                            ocp = opool.tile([128, 512], F16, tag="ocp",
                                             name="ocp")
                            ocp = ocp[:, 0:tlen]
                            if final and fc % 2 == 1:
                                nc.scalar.copy(ocp, ps)
                            else:
                                nc.vector.tensor_copy(ocp, ps)
                            if final:
                                oeng = nc.sync if fc % 2 == 0 else nc.scalar
                            else:
                                oeng = nc.gpsimd
                            oeng.dma_start(
                                out=outT[pair, bass.ts(fc, 128), tsl],
                                in_=ocp)

                carry = {}

                def emit_score(qc, kc, toff, i, ptiles):
                    sc = scp.tile([128, 1024], F32, tag="sc", name="sc")
                    nc.tensor.matmul(
                        sc[:, 0:512],
                        qk[0:64, kc, bass.ts(i, 128)],
                        qk[0:64, qc, bass.ds(toff, 512)],
                        start=True, stop=True)
                    nc.tensor.matmul(
                        sc[:, 512:1024],
                        qk[64:128, kc, bass.ts(i, 128)],
                        qk[64:128, qc, bass.ds(toff, 512)],
                        start=True, stop=True)
                    p = ppool.tile([128, 1024], F16, tag="p", name="p")
                    nc.scalar.activation(p, sc, EXP, scale=SCALING)
                    ptiles[i] = p

                def emit_pair(pair, tasks=(), nxt=None):
                    hA, hB = 2 * pair, 2 * pair + 1
                    qc = pair       # Q chunk of this pair
                    kc = 2 + pair   # K chunk
                    for tq in range(4):
                        toff = tq * 512
                        tq_tasks = list(tasks[tq]) if tq < len(tasks) else []
                        pvA = pvp.tile([128, 512], F32, tag="pvA")
                        pvB = pvp.tile([128, 512], F32, tag="pvB")
                        # software-pipelined: scores/exp for i, P@V for
                        # i-1, so the PE streams scores while ACT runs.
                        ptiles = {}
                        start_i = 0
                        if carry.get("key") == (pair, tq):
                            ptiles.update(carry["ptiles"])
                            start_i = carry["n"]
                            carry.clear()
                            # strips 0-1 were hoisted across the boundary;
                            # keep step 0 task-free so the PE reaches this
                            # window's first new scores (sc2/sc3) without
                            # a task in front of them.
                            tq_tasks.insert(0, [])
                        with nc.named_scope(f"scores{pair}_{tq}"):
                            # two iterations per step: one scores->PV
                            # array handoff per TWO strips, the four
                            # P@V matmuls chaining at stream rate, and one
                            # micro-task absorbed into the step slack. The
                            # exp ACT (2 strips in flight, scp/ppool
                            # double-buffered) stays the pacer.
                            for ib in range(0, NSCH + 2, 2):
                                for i in (ib, ib + 1):
                                    if i >= NSCH or i < start_i:
                                        continue
                                    emit_score(qc, kc, toff, i, ptiles)
                                if tq_tasks:
                                    for t in tq_tasks.pop(0):
                                        t()
                                if ib == NSCH:
                                    # hoist the NEXT window's first score
                                    # pair + exps ahead of this window's
                                    # final P@V drain so the exp stream
                                    # crosses the boundary gap-free.
                                    if tq < 3:
                                        nkey = (pair, tq + 1)
                                        nq, nk, nt = qc, kc, (tq + 1) * 512
                                    elif nxt is not None:
                                        nkey = (nxt, 0)
                                        nq, nk, nt = nxt, 2 + nxt, 0
                                    else:
                                        nkey = None
                                    if nkey is not None:
                                        cpt = {}
                                        for i2 in (0, 1):
                                            emit_score(nq, nk, nt, i2, cpt)
                                        carry["key"] = nkey
                                        carry["ptiles"] = cpt
                                        carry["n"] = 2
                                for i in (ib - 2, ib - 1):
                                    if i < 0 or i >= NSCH:
                                        continue
                                    pp = ptiles.pop(i)
                                    nc.tensor.matmul(
                                        pvA,
                                        va[:, i,
                                           hA * 65:hA * 65 + 128],
                                        pp[:, 0:512],
                                        start=(i == 0),
                                        stop=(i == NSCH - 1))
                                    nc.tensor.matmul(
                                        pvB,
                                        va[:, i,
                                           hB * 65:hB * 65 + 128],
                                        pp[:, 512:1024],
                                        start=(i == 0),
                                        stop=(i == NSCH - 1))
                            for step in tq_tasks:
                                for t in step:
                                    t()
                        # finalize: stage unnormalized P@V + sums to SBUF
                        # (frees PSUM), reciprocal via partition shuffle,
                        # stride-0 DMA broadcast, normalize. With out_proj
                        # deferred TWO windows, this whole chain runs in the
                        # shadow of the next window — no PE instruction
                        # waits on it until two windows later.
                        last = (pair == 1 and tq == 3)
                        if last:
                            # tail: DMA-free finalize. Stage pv rows 0-95
                            # via the (now idle) ACT; DVE 32x32 block
                            # transposes put the denominator row into a
                            # strided column ([32,16] -> fast reciprocal),
                            # a reverse block transpose rebuilds the
                            # contiguous 1/denom row, and a PE ones-matmul
                            # broadcasts it into PSUM for the normalize.
                            # Saves the ~6us recscr DRAM bounce latency.
                            with nc.named_scope("fintail"):
                                un = unpool.tile([64, 2, 512], F32,
                                                 tag="unt", bufs=1,
                                                 name="untail")
                                rsq = fpool.tile([32, 2, 512], F16,
                                                 tag="rsq", bufs=1,
                                                 name="rsq")
                                nc.vector.memset(rsq, 1.0)
                                rowT = fpool.tile([32, 2, 512], F16,
                                                  tag="rowT", bufs=1,
                                                  name="rowT")
                                bc = scp.tile([128, 1024], F32, tag="sc",
                                              name="bctail")
                                tp = fpool.tile([32, 2, 512], F32,
                                                tag="tpose", bufs=1,
                                                name="tp")
                                # transpose the denominator rows straight
                                # out of PSUM (DVE can read PSUM), in
                                # parallel with the ACT staging copies.
                                nc.vector.transpose(
                                    tp[:, 0, :], pvA[64:96, :])
                                nc.vector.transpose(
                                    tp[:, 1, :], pvB[64:96, :])
                                nc.scalar.copy(un[:, 0, :], pvA[0:64, :])
                                nc.scalar.copy(un[:, 1, :], pvB[0:64, :])
                                with nc.allow_low_precision(
                                        reason="fp16 1/denom feeds "
                                               "fp16 normalize"):
                                    nc.vector.reciprocal(
                                        rsq.rearrange(
                                            "p h (b j) -> p h b j",
                                            j=32)[:, :, :, 0:1],
                                        tp.rearrange(
                                            "p h (b j) -> p h b j",
                                            j=32)[:, :, :, 0:1])
                                nc.vector.transpose(rowT, rsq)
                                for h in range(2):
                                    nc.tensor.matmul(
                                        bc[0:64, bass.ts(h, 512)],
                                        ones64[0:1, 0:64],
                                        rowT[0:1, h, :],
                                        start=True, stop=True)
                                    nc.vector.tensor_mul(
                                        attn[h * 64:h * 64 + 64, pair,
                                             bass.ds(toff, 512)],
                                        un[0:64, h, :],
                                        bc[0:64, bass.ts(h, 512)])
                        else:
                            with nc.named_scope(f"fin{pair}_{tq}"):
                                un = unpool.tile([65, 2, 512], F32,
                                                 tag="un")
                                nc.vector.tensor_copy(
                                    un[:, 0, :], pvA[0:65, :])
                                nc.vector.tensor_copy(
                                    un[:, 1, :], pvB[0:65, :])
                                recin = fpool.tile([128, 8], F32,
                                                   tag="recin")
                                nc.sync.dma_start(
                                    out=recin, in_=un[64:65, :, :])
                                recw = fpool.tile([128, 8], F32, tag="recw")
                                nc.vector.reciprocal(recw, recin)
                                nc.sync.dma_start(
                                    out=recscr[tq % 2], in_=recw)
                                for h in range(2):
                                    prt = h * 64
                                    bcs = opool.tile([64, 512], F32,
                                                     tag="bcs")
                                    nc.sync.dma_start(
                                        out=bcs,
                                        in_=recscr[tq % 2, h:h + 1,
                                                   :].to_broadcast(
                                                       [64, 512]))
                                    nc.vector.tensor_mul(
                                        attn[prt:prt + 64, pair,
                                             bass.ds(toff, 512)],
                                        un[0:64, h, :],
                                        bcs)

                # ---- upfront prefix: only what the first strip-steps of
                # window (0,0) strictly need (K2 strips 0-3, Q0 t-halves
                # 0-1, V chunks 0-5); everything else is absorbed into the
                # windows as per-step tasks, each step list sized to the
                # per-step PE slack under the 1.2us/strip exp pace and
                # ordered to meet its consumer's strip deadline.
                emit_v(range(4))
                emit_qk(2, range(1))
                emit_qk(0, range(1))
                o00 = t_oproj(0, 0)
                o01 = t_oproj(0, 1)
                o02 = t_oproj(0, 2)
                o03 = t_oproj(0, 3)
                o10 = t_oproj(1, 0)
                o11 = t_oproj(1, 1)
                o12 = t_oproj(1, 2)
                emit_pair(0, nxt=1, tasks=[
                    [[t_qk(2, 1)], [t_v(4), t_v(5)],
                     [t_qk(2, 2)], [t_v(6), t_v(7)],
                     [t_qk(2, 3), t_v(8)], [t_v(9), t_v(10)],
                     [t_v(11), t_qk(0, 1)], [t_v(12), t_v(13)],
                     [t_v(14), t_v(15)]],
                    [s for t in (t_qkh(0, 2), t_qkh(1, 0), t_qkh(1, 1))
                     for s in ([t[0]], [t[1]])],
                    [o00[0:1], o00[1:2], o00[2:3], o00[3:4]] +
                    [s for t in (t_qkh(0, 3), t_qkh(3, 0))
                     for s in ([t[0]], [t[1]])],
                    [o01[0:1], o01[1:2], o01[2:3], o01[3:4]] +
                    [s for t in (t_qkh(3, 1), t_qkh(1, 2))
                     for s in ([t[0]], [t[1]])],
                ])
                emit_pair(1, nxt=None, tasks=[
                    [s for t in (t_qkh(3, 2), t_qkh(3, 3))
                     for s in ([t[0]], [t[1]])] +
                    [o02[0:1], o02[1:2], o02[2:3], o02[3:4]],
                    [s for t in (t_qkh(1, 3),) for s in ([t[0]], [t[1]])] +
                    [o03[0:1], o03[1:2], o03[2:3], o03[3:4]],
                    [o10[0:1], o10[1:2], o10[2:3], o10[3:4]],
                    [o11[0:1], o11[1:2], o11[2:3], o11[3:4],
                     o12[0:1], o12[1:2], o12[2:3], o12[3:4]],
                ])
                emit_oproj(1, 3, final=True)
    _split_excess_waits(nc)
    return nc


_NC_CACHE = None


def _get_nc():
    global _NC_CACHE
    if _NC_CACHE is None:
        _NC_CACHE = _build_nc()
    return _NC_CACHE


def kernel(x, in_proj_weight, in_proj_bias, out_proj_weight, out_proj_bias,
           _run_kwargs=None, _capture=None):
    x = np.asarray(x, dtype=np.float32)
    in_proj_weight = np.asarray(in_proj_weight, dtype=np.float32)
    in_proj_bias = np.asarray(in_proj_bias, dtype=np.float32)
    out_proj_weight = np.asarray(out_proj_weight, dtype=np.float32)
    out_proj_bias = np.asarray(out_proj_bias, dtype=np.float32)

    nc = _get_nc()
    xTb = [np.ascontiguousarray(x[:, b, :].T.astype(np.float16))
           for b in range(B)]

    in_maps = []
    for c in range(N_CORES):
        b = c // 4
        h0 = (c % 4) * 4
        rows = slice(h0 * D, h0 * D + 4 * D)
        wq = in_proj_weight[0:E][rows]          # [256, 1024]
        wk = in_proj_weight[E:2 * E][rows]
        wv_ = in_proj_weight[2 * E:3 * E][rows]
        wqkT = np.ascontiguousarray(
            np.concatenate([wq, wk], axis=0).T.astype(np.float16))
        wvT = np.ascontiguousarray(wv_.T.astype(np.float16))
        woutT = np.ascontiguousarray(
            out_proj_weight[:, rows].T.astype(np.float16))
        bqk = np.concatenate(
            [in_proj_bias[0:E][rows], in_proj_bias[E:2 * E][rows]])
        bias_qk = np.ascontiguousarray(bqk.reshape(4, 128).T)
        bias_v = in_proj_bias[2 * E:3 * E][rows].reshape(1, 256)
        in_maps.append({
            "xT": xTb[b],
            "wqkT": wqkT,
            "wvT": wvT,
            "woutT": woutT,
            "bias_qk": bias_qk,
            "bias_v": np.ascontiguousarray(bias_v),
        })

    res = run_bass_kernel_spmd(nc, in_maps, core_ids=list(range(N_CORES)),
                               **(_run_kwargs or {}))
    if _capture is not None:
        _capture["res"] = res

    out = np.zeros((S, B, E), dtype=np.float32)
    for c in range(N_CORES):
        b = c // 4
        o = res.results[c]["outT"]
        out[:, b, :] += o[0].T.astype(np.float32)
        out[:, b, :] += o[1].T.astype(np.float32)
    out += out_proj_bias
    return out

